# revision 14
# baseline (speedup 1.0000x reference)
"""Trainium2 Bass kernel for nn_APP_30227979829323 (dense transformer block).

reference:
    qp = q @ Wq + bq
    kp = LN(k; g,b) @ Wk + bk          (layernorm folded into the projection)
    vp = LN(v; g,b) @ Wv + bv
    scores = qh @ kh^T  (per head)
    attn = softmax(gelu(scores) * Dh^-0.5)
    out = L2normalize((attn @ vh) @ Wo + bo)
    returns (out, attn)

Sharding: 8 cores = (batch b in 0..3) x (query-row half in 0..1).
Each core computes all 16 heads for its 512 query rows; k/v projections are
computed redundantly within each batch pair (no collectives).

On-device layout is "transposed" (feature-on-partition); scores are produced
transposed [c_k, c_q] so the attn@v matmul consumes them directly.  The
attention matrix is written transposed and the host assembles [B, H, C, C].
"""

import json
import os
import numpy as np
from contextlib import ExitStack

import concourse.bass as bass
import concourse.tile as tile
import concourse.mybir as mybir
from concourse.bass_utils import run_bass_kernel_spmd

F16 = mybir.dt.float16
F32 = mybir.dt.float32
AF = mybir.ActivationFunctionType

B, C, D = 4, 1024, 1024
H, DH = 16, 64
NCORES = 8
CQ = C // 2          # query rows per core
P = 128
NCH = D // P         # 8 feature chunks of 128
KC = C // P          # 8 key chunks
SCALE = DH ** -0.5
LN_EPS = 1e-5

# ---------------------------------------------------------------------------
# walrus workaround: this toolchain rejects instructions carrying more than
# one semaphore wait ("Too many sync wait commands").  Split excess waits onto
# preceding EventSemaphore instructions on the same engine.
_WAIT_LIM = 1


def _split_sync_waits(bir: dict, lim: int = _WAIT_LIM) -> None:
    for fn in bir["functions"]:
        for bb in fn["blocks"]:
            out = []
            n = 0
            for inst in bb["instructions"]:
                si = inst.get("sync_info")
                if si and inst.get("engine") not in (None, "Unassigned"):
                    ow = si.get("on_wait") or []
                    if len(ow) > lim:
                        extras = ow[: len(ow) - lim]
                        si["on_wait"] = ow[len(ow) - lim:]
                        for i in range(0, len(extras), lim):
                            n += 1
                            out.append({
                                "debug": inst.get("debug", 0),
                                "engine": inst["engine"],
                                "ins": [], "outs": [],
                                "name": f"{inst['name']}-ws{n}",
                                "opcode": "EventSemaphore",
                                "sync_info": {"on_update": [],
                                              "on_wait": extras[i:i + lim]},
                            })
                out.append(inst)
            bb["instructions"] = out


_orig_to_json_bytes = bass.Bass.to_json_bytes
_patched = False


def _install_waitsplit():
    global _patched
    if _patched:
        return
    _patched = True

    def to_json_bytes_patched(self):
        d = json.loads(_orig_to_json_bytes(self))
        _split_sync_waits(d)
        return json.dumps(d).encode()

    bass.Bass.to_json_bytes = to_json_bytes_patched


# ---------------------------------------------------------------------------
_uid = [0]


def _tg() -> str:
    _uid[0] += 1
    return f"u{_uid[0]}"


def build_nc():
    _install_waitsplit()
    nc = bass.Bass("TRN2", target_bir_lowering=False, debug=False,
                   num_devices=NCORES)

    # ---- I/O ----
    qT = nc.dram_tensor("qT", [D, CQ], F16, kind="ExternalInput").ap()
    kT = nc.dram_tensor("kT", [D, C], F16, kind="ExternalInput").ap()
    vT = nc.dram_tensor("vT", [D, C], F16, kind="ExternalInput").ap()
    Wq = nc.dram_tensor("Wq", [D, D], F16, kind="ExternalInput").ap()
    Wk = nc.dram_tensor("Wk", [D, D], F16, kind="ExternalInput").ap()
    Wv = nc.dram_tensor("Wv", [D, D], F16, kind="ExternalInput").ap()
    Wo = nc.dram_tensor("Wo", [D, D], F16, kind="ExternalInput").ap()
    bq_col = nc.dram_tensor("bq_col", [P, NCH], F32, kind="ExternalInput").ap()
    gk_col = nc.dram_tensor("gk_col", [P, NCH], F32, kind="ExternalInput").ap()
    gv_col = nc.dram_tensor("gv_col", [P, NCH], F32, kind="ExternalInput").ap()
    BkT = nc.dram_tensor("BkT", [P, NCH], F16, kind="ExternalInput").ap()
    BvT = nc.dram_tensor("BvT", [P, NCH], F16, kind="ExternalInput").ap()
    bk_row = nc.dram_tensor("bk_row", [1, D], F32, kind="ExternalInput").ap()
    bv_row = nc.dram_tensor("bv_row", [1, D], F32, kind="ExternalInput").ap()
    bo_row = nc.dram_tensor("bo_row", [1, D], F16, kind="ExternalInput").ap()
    attnT = nc.dram_tensor("attnT", [H, C, CQ], F32, kind="ExternalOutput").ap()
    out_rows = nc.dram_tensor("out_rows", [CQ, D], F32,
                              kind="ExternalOutput").ap()
    debug = bool(os.environ.get("K_DEBUG"))
    if debug:
        qpT_out = nc.dram_tensor("qpT_out", [P, NCH, CQ], F16,
                                 kind="ExternalOutput").ap()
        kpT_out = nc.dram_tensor("kpT_out", [P, NCH, C], F16,
                                 kind="ExternalOutput").ap()
        vp_out = nc.dram_tensor("vp_out", [P, NCH, D], F16,
                                kind="ExternalOutput").ap()

    with tile.TileContext(nc) as tc, ExitStack() as top:
        per = top.enter_context(tc.tile_pool(name="per", bufs=1))
        mm = top.enter_context(tc.tile_pool(name="mm", bufs=4, space="PSUM"))

        qpT = per.tile([P, NCH, CQ], F16, tag=_tg(), name="qpT")
        kpT = per.tile([P, NCH, C], F16, tag=_tg(), name="kpT")
        vp = per.tile([P, NCH, D], F16, tag=_tg(), name="vp")
        Wo_sb = per.tile([P, NCH, D], F16, tag=_tg(), name="Wo_sb")
        bo_sb = per.tile([1, D], F16, tag=_tg(), name="bo_sb")
        ones_row = per.tile([1, P], F16, tag=_tg(), name="ones_row")
        nc.vector.memset(ones_row[:], 1.0)
        onesPP = per.tile([P, P], F16, tag=_tg(), name="onesPP")
        nc.vector.memset(onesPP[:], 1.0)
        nc.sync.dma_start(Wo_sb[:], Wo.rearrange("(c p) e -> p c e", p=P))
        nc.sync.dma_start(bo_sb[:], bo_row[:])

        # ---------------- phase 1: LN stats + projections ----------------
        with ExitStack() as ph1:
            w_in = ph1.enter_context(tc.tile_pool(name="w_in", bufs=1))
            sq_pool = ph1.enter_context(tc.tile_pool(name="sq", bufs=2))
            srow = ph1.enter_context(tc.tile_pool(name="srow", bufs=1))
            ps_st = ph1.enter_context(tc.tile_pool(name="ps_st", bufs=4,
                                                   space="PSUM"))

            qT_sb = w_in.tile([P, NCH, CQ], F16, tag=_tg(), name="qT_sb")
            kT_sb = w_in.tile([P, NCH, C], F16, tag=_tg(), name="kT_sb")
            vT_sb = w_in.tile([P, NCH, C], F16, tag=_tg(), name="vT_sb")
            Wq_sb = w_in.tile([P, NCH, D], F16, tag=_tg(), name="Wq_sb")
            Wk_sb = w_in.tile([P, NCH, D], F16, tag=_tg(), name="Wk_sb")
            Wv_sb = w_in.tile([P, NCH, D], F16, tag=_tg(), name="Wv_sb")
            nc.sync.dma_start(qT_sb[:], qT.rearrange("(c p) e -> p c e", p=P))
            nc.sync.dma_start(kT_sb[:], kT.rearrange("(c p) e -> p c e", p=P))
            nc.sync.dma_start(vT_sb[:], vT.rearrange("(c p) e -> p c e", p=P))
            nc.sync.dma_start(Wq_sb[:], Wq.rearrange("(c p) e -> p c e", p=P))
            nc.sync.dma_start(Wk_sb[:], Wk.rearrange("(c p) e -> p c e", p=P))
            nc.sync.dma_start(Wv_sb[:], Wv.rearrange("(c p) e -> p c e", p=P))
            bq_sb = srow.tile([P, NCH], F32, tag=_tg(), name="bq_sb")
            gk_sb = srow.tile([P, NCH], F32, tag=_tg(), name="gk_sb")
            gv_sb = srow.tile([P, NCH], F32, tag=_tg(), name="gv_sb")
            Bk_sb = srow.tile([P, NCH], F16, tag=_tg(), name="Bk_sb")
            Bv_sb = srow.tile([P, NCH], F16, tag=_tg(), name="Bv_sb")
            nc.sync.dma_start(bq_sb[:], bq_col)
            nc.sync.dma_start(gk_sb[:], gk_col)
            nc.sync.dma_start(gv_sb[:], gv_col)
            nc.sync.dma_start(Bk_sb[:], BkT)
            nc.sync.dma_start(Bv_sb[:], BvT)
            ones1 = srow.tile([P, 1], F16, tag=_tg(), name="ones1")
            nc.vector.memset(ones1[:], 1.0)
            ones_f32 = srow.tile([1, P], F32, tag=_tg(), name="ones_f32")
            nc.vector.memset(ones_f32[:], 1.0)

            proj_specs = []
            for nm, xT_sb, W_sb, g_sb, Beta_sb, b_dram in (
                ("k", kT_sb, Wk_sb, gk_sb, Bk_sb, bk_row),
                ("v", vT_sb, Wv_sb, gv_sb, Bv_sb, bv_row),
            ):
                # per-column sums of x and x^2
                ps_sum = [ps_st.tile([1, 512], F32, tag="ps_stat",
                                     name=f"pssum{nm}{i}") for i in range(2)]
                ps_sq = [ps_st.tile([1, 512], F32, tag="ps_stat",
                                    name=f"pssq{nm}{i}") for i in range(2)]
                for dc in range(NCH):
                    x2 = sq_pool.tile([P, C], F16, tag="x2", name=f"x2{nm}{dc}")
                    nc.vector.tensor_mul(x2[:], xT_sb[:, dc, :], xT_sb[:, dc, :])
                    for ct in range(2):
                        nc.tensor.matmul(ps_sum[ct][:], ones1[:],
                                         xT_sb[:, dc, bass.ts(ct, 512)],
                                         start=(dc == 0), stop=(dc == NCH - 1))
                        nc.tensor.matmul(ps_sq[ct][:], ones1[:],
                                         x2[:, bass.ts(ct, 512)],
                                         start=(dc == 0), stop=(dc == NCH - 1))
                # f32 row working set: A=sum->mean->std, B=sumsq->ex2->r,
                # Cr=var, Dr=b (slots shared across the k/v iterations)
                rowA = srow.tile([1, C], F32, tag="rowA", name=f"rowA{nm}")
                rowB = srow.tile([1, C], F32, tag="rowB", name=f"rowB{nm}")
                rowC = srow.tile([1, C], F32, tag="rowC", name=f"rowC{nm}")
                for ct in range(2):
                    nc.vector.tensor_scalar_mul(rowA[:, bass.ts(ct, 512)],
                                                ps_sum[ct][:], 1.0 / D)
                    nc.vector.tensor_scalar_mul(rowB[:, bass.ts(ct, 512)],
                                                ps_sq[ct][:], 1.0 / D)
                mean = rowA
                nc.vector.tensor_mul(rowC[:], mean[:], mean[:])
                nc.vector.tensor_sub(rowC[:], rowB[:], rowC[:])   # var
                nc.vector.tensor_scalar_add(rowC[:], rowC[:], LN_EPS)
                negm = srow.tile([1, C], F16, tag=_tg(), name=f"negm{nm}")
                nc.vector.tensor_scalar_mul(negm[:], mean[:], -1.0)
                std = rowA  # overwrite mean (negm extracted)
                nc.scalar.activation(std[:], rowC[:], AF.Sqrt)
                std_h = srow.tile([1, C], F16, tag=_tg(), name=f"stdh{nm}")
                nc.vector.tensor_copy(std_h[:], std[:])
                r_row = rowB  # overwrite ex2
                nc.vector.reciprocal(r_row[:], std[:])

                # w = beta^T W + b (before g-scaling); u = 1^T (g*W)
                ps_w = [ps_st.tile([1, 512], F32, tag="ps_stat",
                                   name=f"psw{nm}{i}") for i in range(2)]
                for dc in range(NCH):
                    for ct in range(2):
                        nc.tensor.matmul(ps_w[ct][:], Beta_sb[:, dc:dc + 1],
                                         W_sb[:, dc, bass.ts(ct, 512)],
                                         start=(dc == 0), stop=(dc == NCH - 1))
                w_row = srow.tile([1, D], F32, tag="rowD", name=f"w_row{nm}")
                nc.sync.dma_start(w_row[:], b_dram)
                w_rowh = srow.tile([1, D], F16, tag=_tg(), name=f"w_rowh{nm}")
                for ct in range(2):
                    nc.vector.tensor_add(w_row[:, bass.ts(ct, 512)],
                                         ps_w[ct][:], w_row[:, bass.ts(ct, 512)])
                nc.vector.tensor_copy(w_rowh[:], w_row[:])
                for dc in range(NCH):
                    nc.vector.tensor_scalar_mul(W_sb[:, dc, :], W_sb[:, dc, :],
                                                g_sb[:, dc:dc + 1])
                ps_u = [ps_st.tile([1, 512], F32, tag="ps_stat",
                                   name=f"psu{nm}{i}") for i in range(2)]
                for dc in range(NCH):
                    for ct in range(2):
                        nc.tensor.matmul(ps_u[ct][:], ones1[:],
                                         W_sb[:, dc, bass.ts(ct, 512)],
                                         start=(dc == 0), stop=(dc == NCH - 1))
                u_rowh = srow.tile([1, D], F16, tag=_tg(), name=f"u_rowh{nm}")
                for ct in range(2):
                    nc.vector.tensor_copy(u_rowh[:, bass.ts(ct, 512)],
                                          ps_u[ct][:])
                rB = None
                r_col = None
                if nm == "k":
                    # rB[p, c] = r[c] broadcast across partitions (PE outer)
                    rB = srow.tile([P, C], F32, tag=_tg(), name=f"rB{nm}")
                    for ct in range(2):
                        ps_rb = mm.tile([P, 512], F32, tag="mm",
                                        name=f"psrb{nm}{ct}")
                        nc.tensor.matmul(ps_rb[:], ones_f32[:],
                                         r_row[:, bass.ts(ct, 512)],
                                         start=True, stop=True)
                        nc.vector.tensor_copy(rB[:, bass.ts(ct, 512)], ps_rb[:])
                else:
                    # r_col[p, cc] = r[cc*128+p] via K=1,N=1 PE outers
                    r_col = srow.tile([P, NCH], F32, tag=_tg(), name=f"r_col{nm}")
                    for cc in range(NCH):
                        ps_rc = ps_st.tile([P, 1], F32, tag="ps_stat",
                                           name=f"psrc{nm}{cc}")
                        nc.tensor.matmul(ps_rc[:], r_row[0:1, bass.ts(cc, P)],
                                         ones_f32[0:1, 0:1],
                                         start=True, stop=True)
                        nc.vector.tensor_copy(r_col[:, cc:cc + 1], ps_rc[:])
                proj_specs.append(dict(xT=xT_sb, W=W_sb, negm=negm, u=u_rowh,
                                       std_h=std_h, w_rowh=w_rowh, r_col=r_col,
                                       rB=rB))

            # qp^T[e, c] : plain projection + per-partition bias
            for ec in range(NCH):
                ps = mm.tile([P, 512], F32, tag="mm", name=f"psq{ec}")
                for dc in range(NCH):
                    nc.tensor.matmul(ps[:, :CQ], Wq_sb[:, dc, bass.ts(ec, P)],
                                     qT_sb[:, dc, :], start=(dc == 0),
                                     stop=(dc == NCH - 1))
                nc.vector.tensor_scalar_add(qpT[:, ec, :], ps[:, :CQ],
                                            bq_sb[:, ec:ec + 1])

            # kp^T[e, c] = rB * (W'^T kT - u x m) + w_col
            sk = proj_specs[0]
            for ec in range(NCH):
                for ct in range(2):
                    ps = mm.tile([P, 512], F32, tag="mm", name=f"psk{ec}{ct}")
                    for dc in range(NCH):
                        nc.tensor.matmul(ps[:], sk["W"][:, dc, bass.ts(ec, P)],
                                         sk["xT"][:, dc, bass.ts(ct, 512)],
                                         start=(dc == 0), stop=False)
                    nc.tensor.matmul(ps[:], sk["u"][0:1, bass.ts(ec, P)],
                                     sk["negm"][0:1, bass.ts(ct, 512)],
                                     start=False, stop=False)
                    nc.tensor.matmul(ps[:], sk["w_rowh"][0:1, bass.ts(ec, P)],
                                     sk["std_h"][0:1, bass.ts(ct, 512)],
                                     start=False, stop=True)
                    dst = kpT[:, ec, bass.ts(ct, 512)]
                    nc.vector.tensor_mul(dst, ps[:], sk["rB"][:, bass.ts(ct, 512)])

            # vp[c, e] = r_col * (vT^T W' - m x u + std x w)   (normal layout)
            sv = proj_specs[1]
            for cc in range(NCH):
                for et in range(2):
                    ps = mm.tile([P, 512], F32, tag="mm", name=f"psv{cc}{et}")
                    for dc in range(NCH):
                        nc.tensor.matmul(ps[:], sv["xT"][:, dc, bass.ts(cc, P)],
                                         sv["W"][:, dc, bass.ts(et, 512)],
                                         start=(dc == 0), stop=False)
                    nc.tensor.matmul(ps[:], sv["negm"][0:1, bass.ts(cc, P)],
                                     sv["u"][0:1, bass.ts(et, 512)],
                                     start=False, stop=False)
                    nc.tensor.matmul(ps[:], sv["std_h"][0:1, bass.ts(cc, P)],
                                     sv["w_rowh"][0:1, bass.ts(et, 512)],
                                     start=False, stop=True)
                    dst = vp[:, cc, bass.ts(et, 512)]
                    nc.vector.tensor_scalar_mul(dst, ps[:],
                                                sv["r_col"][:, cc:cc + 1])

        if debug:
            nc.sync.dma_start(qpT_out, qpT[:])
            nc.sync.dma_start(kpT_out, kpT[:])
            nc.sync.dma_start(vp_out, vp[:])

        # ---------------- phase 2: attention ----------------
        with ExitStack() as ph2:
            pt_pool = ph2.enter_context(tc.tile_pool(name="pt", bufs=34))
            rec_pool = ph2.enter_context(tc.tile_pool(name="rec", bufs=4))
            at_pool = ph2.enter_context(tc.tile_pool(name="at", bufs=6))
            ao_pool = ph2.enter_context(tc.tile_pool(name="ao", bufs=1))
            ps_rs = ph2.enter_context(tc.tile_pool(name="ps_rs", bufs=2,
                                                   space="PSUM"))
            ps_av = ph2.enter_context(tc.tile_pool(name="ps_av", bufs=2,
                                                   space="PSUM"))
            aoT = ao_pool.tile([P, NCH, CQ], F16, tag=_tg(), name="aoT")

            for pair in range(NCH):
                pts = {}
                for h2, poff in ((0, 0), (1, 64)):
                    for kc in range(KC):
                        ps = mm.tile([P, 512], F32, tag="mm",
                                     name=f"pss{pair}{h2}{kc}")
                        nc.tensor.matmul(
                            ps[:, :CQ],
                            kpT[poff:poff + DH, pair, bass.ts(kc, P)],
                            qpT[poff:poff + DH, pair, :],
                            start=True, stop=True, tile_position=(poff, 0))
                        pt = pt_pool.tile([P, CQ], F16, tag="pt",
                                          name=f"pt{pair}{h2}{kc}")
                        nc.scalar.activation(pt[:], ps[:, :CQ], AF.Gelu)
                        pts[(h2, kc)] = pt
                rs_ps = {}
                for h2, poff in ((0, 0), (1, 64)):
                    rs = ps_rs.tile([P, 512], F32, tag="rs",
                                    name=f"rs{pair}{h2}")
                    rs_ps[h2] = rs
                    for kc in range(KC):
                        pt = pts[(h2, kc)]
                        nc.scalar.activation(pt[:], pt[:], AF.Exp, scale=SCALE)
                        nc.tensor.matmul(rs[:, :CQ], onesPP[:], pt[:],
                                         start=(kc == 0), stop=(kc == KC - 1))
                av = ps_av.tile([P, 512], F32, tag="av", name=f"av{pair}")
                recips = {}
                for h2, poff in ((0, 0), (1, 64)):
                    h = 2 * pair + h2
                    rec = rec_pool.tile([P, CQ], F32, tag="rec",
                                        name=f"rec{pair}{h2}")
                    nc.vector.reciprocal(rec[:], rs_ps[h2][:, :CQ])
                    recips[h2] = rec
                    for kc in range(KC):
                        pt = pts[(h2, kc)]
                        nc.tensor.matmul(av[poff:poff + DH, :CQ],
                                         vp[:, kc, bass.ds(h * DH, DH)], pt[:],
                                         start=(kc == 0), stop=(kc == KC - 1),
                                         tile_position=(0, poff))
                        at = at_pool.tile([P, CQ], F32, tag="at",
                                          name=f"at{pair}{h2}{kc}")
                        nc.vector.tensor_mul(at[:], pt[:], rec[:])
                        nc.sync.dma_start(attnT[h, bass.ts(kc, P), :], at[:])
                for h2, poff in ((0, 0), (1, 64)):
                    nc.vector.tensor_mul(aoT[poff:poff + DH, pair, :],
                                         av[poff:poff + DH, :CQ],
                                         recips[h2][poff:poff + DH, :])

            # ------------- phase 3: output projection + L2 -------------
            sq_ps = ph2.enter_context(tc.tile_pool(name="sqs", bufs=4))
            o_pool = ph2.enter_context(tc.tile_pool(name="op", bufs=4))
            junk = sq_ps.tile([P, 512], F16, tag="junk", name="junk", bufs=1)
            for cm in range(CQ // P):
                pss = []
                ssqs = []
                for ft in range(2):
                    ps = mm.tile([P, 512], F32, tag="mm", name=f"pso{cm}{ft}")
                    for pair in range(NCH):
                        nc.tensor.matmul(ps[:], aoT[:, pair, bass.ts(cm, P)],
                                         Wo_sb[:, pair, bass.ts(ft, 512)],
                                         start=(pair == 0), stop=False)
                    nc.tensor.matmul(ps[:], ones_row[0:1, :],
                                     bo_sb[0:1, bass.ts(ft, 512)],
                                     start=False, stop=True)
                    ssq = sq_ps.tile([P, 1], F32, tag="ssq", name=f"ssq{cm}{ft}")
                    nc.scalar.activation(junk[:], ps[:], AF.Square,
                                         accum_out=ssq[:])
                    pss.append(ps)
                    ssqs.append(ssq)
                tot = sq_ps.tile([P, 1], F32, tag="ssq", name=f"tot{cm}")
                nc.vector.tensor_add(tot[:], ssqs[0][:], ssqs[1][:])
                nc.vector.tensor_scalar_max(tot[:], tot[:], 1e-24)
                rinv = sq_ps.tile([P, 1], F32, tag="ssq", name=f"rinv{cm}")
                nc.vector.reciprocal(rinv[:], tot[:])
                rnorm = sq_ps.tile([P, 1], F32, tag="ssq", name=f"rnorm{cm}")
                nc.scalar.activation(rnorm[:], rinv[:], AF.Sqrt)
                for ft in range(2):
                    o = o_pool.tile([P, 512], F32, tag="o", name=f"o{cm}{ft}")
                    nc.vector.tensor_scalar_mul(o[:], pss[ft][:], rnorm[:])
                    nc.sync.dma_start(
                        out_rows[bass.ts(cm, P), bass.ts(ft, 512)], o[:])

    return nc


_NC_CACHE = {}


def _get_nc():
    if "nc" not in _NC_CACHE:
        _NC_CACHE["nc"] = build_nc()
    return _NC_CACHE["nc"]


def _col(x):
    """[D] fp32 -> [P, NCH] per-partition column form."""
    return np.ascontiguousarray(np.asarray(x).reshape(NCH, P).T)


def make_in_maps(q, k, v, ln_k_g, ln_k_b, ln_v_g, ln_v_b,
                 Wq, bq, Wk, bk, Wv, bv, Wo, bo):
    f16 = np.float16
    f32 = np.float32
    shared = dict(
        Wq=np.ascontiguousarray(Wq, dtype=f16),
        Wk=np.ascontiguousarray(Wk, dtype=f16),
        Wv=np.ascontiguousarray(Wv, dtype=f16),
        Wo=np.ascontiguousarray(Wo, dtype=f16),
        bq_col=_col(np.asarray(bq, f32)),
        gk_col=_col(np.asarray(ln_k_g, f32)),
        gv_col=_col(np.asarray(ln_v_g, f32)),
        BkT=_col(np.asarray(ln_k_b, f32)).astype(f16),
        BvT=_col(np.asarray(ln_v_b, f32)).astype(f16),
        bk_row=np.asarray(bk, f32).reshape(1, D).copy(),
        bv_row=np.asarray(bv, f32).reshape(1, D).copy(),
        bo_row=np.asarray(bo, f32).astype(f16).reshape(1, D).copy(),
    )
    in_maps = []
    for core in range(NCORES):
        b, half = divmod(core, 2)
        rows = slice(half * CQ, (half + 1) * CQ)
        m = dict(shared)
        m["qT"] = np.ascontiguousarray(np.asarray(q[b, rows]).T, dtype=f16)
        m["kT"] = np.ascontiguousarray(np.asarray(k[b]).T, dtype=f16)
        m["vT"] = np.ascontiguousarray(np.asarray(v[b]).T, dtype=f16)
        in_maps.append(m)
    return in_maps


def assemble(results):
    out = np.empty((B, C, D), np.float32)
    attn = np.empty((B, H, C, C), np.float32)
    for core in range(NCORES):
        b, half = divmod(core, 2)
        rows = slice(half * CQ, (half + 1) * CQ)
        out[b, rows] = results[core]["out_rows"]
        attn[b, :, rows, :] = results[core]["attnT"].transpose(0, 2, 1)
    return out, attn


def kernel(**inputs):
    nc = _get_nc()
    in_maps = make_in_maps(**inputs)
    res = run_bass_kernel_spmd(nc, in_maps, core_ids=list(range(NCORES)))
    return assemble(res.results)
